# revision 13
# baseline (speedup 1.0000x reference)
"""Causal self-attention Trainium2 kernel (B=4, T=2048, C=1024, H=16, D=64).

Sharding (8 cores): core c -> batch b = c//2, head-group g = c%2 (8
contiguous heads = 512 contiguous channels). Each core computes a partial
projection output for its batch; the host sums the two partials per batch
and adds b_proj.

Per-core pipeline (all on one NeuronCore, Tile-scheduled):
  1. qkv^T: Q^T,K^T computed directly transposed via matmul(lhsT=W, rhs=x^T)
     as 8 rotating [128, 2048] tiles (head-pair p in tile p: head 2p on
     partitions 0:64, head 2p+1 on 64:128).  V computed in natural layout
     (t on partitions), stored bf16 padded [128, 8, 65] with a ones column
     per head so the PV matmul also produces softmax denominators.
  2. Attention per (head-pair, q-tile 512): S^T tiles [128k, 512q] via two
     K=64 matmuls on PE row-groups 0-1/2-3 (concurrent); P^T = exp(S^T/8)
     on ACT (scores are O(1): no max subtraction needed); causal masking by
     a bf16 0/1 multiply on diagonal-straddling tiles only; O^T_aug[65,512]
     accumulated over k-tiles in PSUM; normalize by reciprocal of row 64 +
     PE outer-product broadcast; write y^T tiles [128, 2048].
  3. proj: out_part = y @ W_proj_rows via lhsT=y^T.
"""

import os
from contextlib import ExitStack

import numpy as np

B, T, C = 4, 2048, 1024
H = 16
D = 64
G_HEADS = 8          # heads per core
GC = G_HEADS * D     # 512 channels per core
N_CORES = 8
KT = 128             # k tile (partition dim of S^T)
QT = 512             # q tile (free dim of S^T)
NQT = T // QT        # 4
NCT = C // 128       # 8 contraction tiles
NTT = T // 128       # 16 t tiles
VW = D + 1           # 65: head slot width in padded V

# This container's walrus codegen supports a limited number of sync waits
# per instruction (struct-dependent); Tile emits up to ~11 (kernel-tail
# drain). Hoist excess waits onto same-engine no-ops inserted just before
# the instruction.
_MAX_WAITS = 1
_MAX_WAITS_BY_TYPE = {}


def _split_excess_waits(nc, mybir):
    for f in nc.m.functions:
        for bb in f.blocks:
            fixes = []
            for idx, inst in enumerate(bb.instructions):
                si = inst.sync_info
                lim = _MAX_WAITS_BY_TYPE.get(type(inst).__name__, _MAX_WAITS)
                if si is not None and len(si.on_wait) > lim:
                    fixes.append((idx, inst, lim))
            for idx, inst, lim in reversed(fixes):
                waits = list(inst.sync_info.on_wait)
                keep = waits[:lim]
                excess = waits[lim:]
                inst.sync_info.on_wait = keep
                nops = []
                for ci in range(0, len(excess), _MAX_WAITS):
                    nop = mybir.InstNoOp(
                        name=f"{inst.name}-wsplit{ci}", ins=[], outs=[])
                    nop.engine = inst.engine
                    nop.sync_info = mybir.SyncInfo(
                        on_wait=excess[ci:ci + _MAX_WAITS], on_update=[])
                    nc.register_instruction(nop, overwrite=True)
                    nops.append(nop)
                for k, nop in enumerate(nops):
                    bb.instructions.insert(idx + k, nop)


def _build_program():
    import concourse.bass as bass
    import concourse.tile as tile
    from concourse import mybir

    f32 = mybir.dt.float32
    bf16 = mybir.dt.bfloat16

    nc = bass.Bass("TRN2", target_bir_lowering=False, debug=False,
                   num_devices=N_CORES)

    xT_d = nc.dram_tensor("xT", [C, T], bf16, kind="ExternalInput").ap()
    wqk_d = nc.dram_tensor("wqk", [C, 2 * GC], bf16, kind="ExternalInput").ap()
    wv_d = nc.dram_tensor("wv", [C, GC], bf16, kind="ExternalInput").ap()
    bqk_d = nc.dram_tensor("bqk", [2 * GC, 1], f32, kind="ExternalInput").ap()
    bvb_d = nc.dram_tensor("bv_bcast", [128, GC], f32, kind="ExternalInput").ap()
    masks_d = nc.dram_tensor("masks", [4, KT, QT], bf16, kind="ExternalInput").ap()
    wp_d = nc.dram_tensor("wp", [GC, C], bf16, kind="ExternalInput").ap()
    out_d = nc.dram_tensor("out_part", [T, C], f32, kind="ExternalOutput").ap()

    scale = float(1.0 / np.sqrt(D))
    EXP = mybir.ActivationFunctionType.Exp

    with tile.TileContext(nc) as tc, ExitStack() as ctx:
        const = ctx.enter_context(tc.tile_pool(name="const", bufs=1))
        xpool = ctx.enter_context(tc.tile_pool(name="xp", bufs=1))
        qkpool = ctx.enter_context(tc.tile_pool(name="qkp", bufs=2))
        vpool = ctx.enter_context(tc.tile_pool(name="vp", bufs=1))
        ypool = ctx.enter_context(tc.tile_pool(name="yp", bufs=1))
        wqpool = ctx.enter_context(tc.tile_pool(name="wqp", bufs=2))
        ppool = ctx.enter_context(tc.tile_pool(name="pp", bufs=2))
        rpool = ctx.enter_context(tc.tile_pool(name="rp", bufs=1))
        psum = ctx.enter_context(tc.tile_pool(name="ps", bufs=2, space="PSUM"))

        # ---- constants ----
        masks_sb = const.tile([KT, 4, QT], bf16)
        for d in range(4):
            nc.sync.dma_start(out=masks_sb[:, d, :], in_=masks_d[d])
        bvb_sb = const.tile([128, GC], f32)
        nc.sync.dma_start(out=bvb_sb, in_=bvb_d)
        ones_sb = const.tile([128, D], bf16)
        nc.vector.memset(ones_sb, 1.0)
        bqk_sb = const.tile([128, 8], f32)  # col m = bias for qk m-tile m
        for m in range(8):
            nc.sync.dma_start(out=bqk_sb[:, m:m + 1],
                              in_=bqk_d[m * 128:(m + 1) * 128])

        # ---- load xT (resident through the qkv matmuls) ----
        xT = []
        for k in range(NCT):
            xt = xpool.tile([128, T], bf16, tag=f"x{k}")
            nc.sync.dma_start(out=xt, in_=xT_d[k * 128:(k + 1) * 128])
            xT.append(xt)

        # ---- V = x @ Wv + bv -> bf16 padded [128, 8, 65] per t-tile ----
        V = []
        with tc.tile_pool(name="wvp", bufs=1) as wvpool:
            wv_sb = []
            for k in range(NCT):
                wvt = wvpool.tile([128, GC], bf16, tag=f"wv{k}")
                nc.sync.dma_start(out=wvt, in_=wv_d[k * 128:(k + 1) * 128])
                wv_sb.append(wvt)
            for m in range(NTT):
                vt = vpool.tile([128, G_HEADS, VW], bf16, tag=f"V{m}")
                V.append(vt)
                nc.vector.memset(vt[:, :, D:VW], 1.0)
                ps = psum.tile([128, GC], f32, tag="s0")
                for k in range(NCT):
                    nc.tensor.matmul(ps, xT[k][:, m * 128:(m + 1) * 128],
                                     wv_sb[k],
                                     start=(k == 0), stop=(k == NCT - 1))
                nc.vector.scalar_tensor_tensor(
                    out=vt[:, :, 0:D],
                    in0=ps.rearrange("p (h d) -> p h d", h=G_HEADS),
                    scalar=1.0,
                    in1=bvb_sb.rearrange("p (h d) -> p h d", h=G_HEADS),
                    op0=mybir.AluOpType.mult,
                    op1=mybir.AluOpType.add,
                )

        # proj weight pool opens after wv released (address reuse)
        wppool = ctx.enter_context(tc.tile_pool(name="wpp", bufs=1))
        opool = ctx.enter_context(tc.tile_pool(name="op", bufs=2))

        # ---- per head-pair: qkT tiles then attention ----
        yT = []
        for p in range(4):
            yt = ypool.tile([128, T], bf16, tag=f"y{p}")
            yT.append(yt)

        for p in range(4):
            pair_qk = []
            for mi, m in enumerate((p, 4 + p)):
                qk = qkpool.tile([128, T], bf16, tag=f"qk{mi}")
                pair_qk.append(qk)
                # prefetch W chunks for this m-tile once, reuse across n
                wqts = []
                for k in range(NCT):
                    wqt = wqpool.tile([128, 128], bf16, tag=f"wq{k}")
                    nc.sync.dma_start(
                        out=wqt, in_=wqk_d[k * 128:(k + 1) * 128,
                                           m * 128:(m + 1) * 128])
                    wqts.append(wqt)
                for n in range(NQT):
                    ps = psum.tile([128, QT], f32, tag="s1")
                    for k in range(NCT):
                        nc.tensor.matmul(ps, wqts[k],
                                         xT[k][:, n * QT:(n + 1) * QT],
                                         start=(k == 0), stop=(k == NCT - 1))
                    nc.vector.tensor_scalar_add(
                        out=qk[:, n * QT:(n + 1) * QT], in0=ps,
                        scalar1=bqk_sb[:, m:m + 1])

            qT, kT_ = pair_qk
            for j in range(NQT):
                n_k = 4 * j + 4  # causal: k-tiles 0 .. 4j+3
                psO = [psum.tile([VW, QT], f32, tag=f"o{h}", name=f"psO{h}")
                       for h in (0, 1)]
                for i in range(n_k):
                    for h in (0, 1):
                        lo, hi = h * D, h * D + D
                        psS = psum.tile([KT, QT], f32, tag=f"s{h}")
                        P = ppool.tile([KT, QT], bf16, tag=f"P{h}")
                        nc.tensor.matmul(
                            psS,
                            kT_[lo:hi, i * KT:(i + 1) * KT],
                            qT[lo:hi, j * QT:(j + 1) * QT])
                        nc.scalar.activation(out=P, in_=psS, func=EXP,
                                             scale=scale)
                        if i >= 4 * j:  # diagonal-straddling tile
                            nc.vector.tensor_mul(
                                P, P, masks_sb[:, i - 4 * j, :])
                        nc.tensor.matmul(
                            psO[h], V[i][:, 2 * p + h, :], P,
                            start=(i == 0), stop=(i == n_k - 1))
                # r = 1/denominator row; broadcast via PE outer product.
                # All DVE ops keep in/out partition bases aligned; head B's
                # result is shifted to partitions 64:128 by an SBUF DMA.
                for h in (0, 1):
                    r = rpool.tile([VW, QT], bf16, tag=f"r{h}")
                    with nc.allow_low_precision(reason="bf16 softmax denom"):
                        nc.vector.reciprocal(out=r[D:VW, :],
                                             in_=psO[h][D:VW, :])
                    rb = psum.tile([D, QT], f32, tag=f"s{h}")
                    nc.tensor.matmul(rb, ones_sb[D:D + 1, :],
                                     r[D:VW, :])
                    rbs = rpool.tile([D, QT], f32, tag=f"rb{h}")
                    nc.vector.tensor_copy(rbs, rb)
                    dst = yT[p][h * D:(h + 1) * D, j * QT:(j + 1) * QT]
                    if h == 0:
                        nc.vector.tensor_mul(dst, psO[0][0:D, :], rbs)
                    else:
                        tmp = rpool.tile([D, QT], bf16, tag="ytmp")
                        nc.vector.tensor_mul(tmp, psO[1][0:D, :], rbs)
                        nc.sync.dma_start(out=dst, in_=tmp)

        # ---- proj ----
        for n in range(2):
            wps = []
            for k in range(4):
                wpt = wppool.tile([128, QT], bf16, tag=f"wp{k}")
                nc.sync.dma_start(
                    out=wpt,
                    in_=wp_d[k * 128:(k + 1) * 128, n * QT:(n + 1) * QT])
                wps.append(wpt)
            for m_t in range(NTT):
                ps = psum.tile([128, QT], f32, tag="s0")
                for k in range(4):
                    nc.tensor.matmul(
                        ps, yT[k][:, m_t * 128:(m_t + 1) * 128],
                        wps[k], start=(k == 0), stop=(k == 3))
                o = opool.tile([128, QT], f32, tag="o")
                nc.vector.tensor_copy(o, ps)
                nc.sync.dma_start(
                    out=out_d[m_t * 128:(m_t + 1) * 128,
                              n * QT:(n + 1) * QT], in_=o)

    _split_excess_waits(nc, mybir)
    return nc


_PROGRAM = None


def _in_maps(x, W_attn, b_attn, W_proj):
    import ml_dtypes
    bf = ml_dtypes.bfloat16
    kk = np.arange(KT)[:, None]
    qq = np.arange(QT)[None, :]
    masks = np.stack([(kk + KT * d <= qq) for d in range(4)])
    masks = masks.astype(ml_dtypes.bfloat16)

    in_maps = []
    for c in range(N_CORES):
        b, g = divmod(c, 2)
        lo = g * GC
        wqk = np.concatenate(
            [W_attn[:, lo:lo + GC], W_attn[:, C + lo:C + lo + GC]], axis=1)
        bqk = np.concatenate(
            [b_attn[lo:lo + GC], b_attn[C + lo:C + lo + GC]])[:, None]
        in_maps.append({
            "xT": np.ascontiguousarray(x[b].T).astype(bf),
            "wqk": np.ascontiguousarray(wqk).astype(bf),
            "wv": np.ascontiguousarray(W_attn[:, 2 * C + lo:2 * C + lo + GC]).astype(bf),
            "bqk": np.ascontiguousarray(bqk),
            "bv_bcast": np.broadcast_to(
                b_attn[2 * C + lo:2 * C + lo + GC], (128, GC)).copy(),
            "masks": masks,
            "wp": np.ascontiguousarray(W_proj[lo:lo + GC, :]).astype(bf),
        })
    return in_maps


def kernel(x, W_attn, b_attn, W_proj, b_proj, **run_kwargs):
    from concourse.bass_utils import run_bass_kernel_spmd

    global _PROGRAM
    if _PROGRAM is None:
        _PROGRAM = _build_program()
    nc = _PROGRAM

    x = np.asarray(x, np.float32)
    W_attn = np.asarray(W_attn, np.float32)
    b_attn = np.asarray(b_attn, np.float32)
    W_proj = np.asarray(W_proj, np.float32)
    b_proj = np.asarray(b_proj, np.float32)

    res = run_bass_kernel_spmd(nc, _in_maps(x, W_attn, b_attn, W_proj),
                               core_ids=list(range(N_CORES)), **run_kwargs)
    parts = [r["out_part"] for r in res.results]
    out = np.stack([parts[2 * b] + parts[2 * b + 1] for b in range(B)])
    out = (out + b_proj).astype(np.float32)
    if run_kwargs:
        kernel.last_results = res
    return out
